# revision 15
# baseline (speedup 1.0000x reference)
"""Trainium2 Bass kernel for nn_Critic (twin-critic LSTM network).

Strategy: pure data parallelism — batch B=1024 split as 128 per core across
8 NeuronCores (128 = SBUF partition count, so batch maps to partitions).

Decomposition (validated in numpy by `decomp_reference`):
  * gate rows reordered (i,f,g,o) -> (i,f,o,g) so one sigmoid covers [0:768]
  * serial LSTMs: per-step gates = lhsT.T @ rhs with
      lhsT = [x_aug_t (17 = 7 state + 9 action + ones) ; h_T (256)]   (K chunks 17/128/128)
      rhs  = [Wih_aug^T ; Whh^T]  (bias folded into the ones row)
    h kept transposed on-chip via PE transpose each step.
  * parallel encoders: same gate matmul with h_T fixed at h0_T; only
    fuse(h_all) is used by the reference, and
      sum_t (z @ f1W.T + f1b) * s0  ==  (sum_t s0_t z_t) @ f1W.T + f1b * S0sum
    The weighted time-sum is computed on PE as diag(s_t) matmuls
    accumulating in PSUM.
  * fuse linears + concat + final LSTM input proj fold into ONE matmul with
    host-precomputed combined weights (1027 contraction rows incl. rank-1
    bias terms via S0sum/S1sum rows).
"""

import os
import sys

import numpy as np

sys.path.insert(0, "/opt/trn_rl_repo")

B, T, A, H, SD = 1024, 64, 9, 256, 7
IN = SD + A  # 16
LONG = 10.0
NCORES = 8
BL = B // NCORES  # 128
TS = T - 1  # 63
G = 4 * H  # 1024
KX = IN + 1  # 17: x features + ones row

F32 = np.float32


def _perm():
    # (i,f,g,o) -> (i,f,o,g)
    return np.r_[0:512, 768:1024, 512:768]


def _enc_weights(p):
    """rhs tensors for one encoder: wx_aug [17,1024], wa [128,1024], wb [128,1024]."""
    perm = _perm()
    wih = np.asarray(p["Wih"], F32)[perm]  # [1024, 16]
    whh = np.asarray(p["Whh"], F32)[perm]  # [1024, 256]
    bias = (np.asarray(p["bih"], F32) + np.asarray(p["bhh"], F32))[perm]  # [1024]
    wx_aug = np.concatenate([wih.T, bias[None, :]], axis=0).astype(F32)  # [17, 1024]
    wa = whh.T[0:128].copy()
    wb = whh.T[128:256].copy()
    return wx_aug, wa, wb


def _final_rhs(params, enc_key):
    """Final-LSTM folded rhs for one critic: fr [128, 8*1024], frt [3, 1024]."""
    perm = _perm()
    L = params["L"]
    wih_L = np.asarray(L["Wih"], F32)[perm]  # [1024, 512]
    whh_L = np.asarray(L["Whh"], F32)[perm]  # [1024, 256]
    bias_L = (np.asarray(L["bih"], F32) + np.asarray(L["bhh"], F32))[perm]  # [1024]
    P1 = wih_L[:, 256:384]  # [1024, 128]
    P2 = wih_L[:, 384:512]
    enc = params[enc_key]
    f1W = np.asarray(enc["f1W"], F32)  # [128, 256]
    f2W = np.asarray(enc["f2W"], F32)
    f1b = np.asarray(enc["f1b"], F32)  # [128]
    f2b = np.asarray(enc["f2b"], F32)
    M1 = P1 @ f1W  # [1024, 256]
    M2 = P2 @ f2W
    rows = np.concatenate(
        [wih_L[:, 0:256].T, M1.T, M2.T, whh_L.T], axis=0
    ).astype(F32)  # [1024, 1024]
    fr = rows.reshape(8, 128, G).transpose(1, 0, 2).reshape(128, 8 * G).copy()
    frt = np.stack([bias_L, P1 @ f1b, P2 @ f2b]).astype(F32)  # [3, 1024]
    return fr, frt


def _prep_shared(params):
    d = {}
    for name, key in [("e1", "l11"), ("e2", "l22"), ("e3", "l33"), ("e4", "l44")]:
        wx, wa, wb = _enc_weights(params[key])
        d[f"wx_{name}"], d[f"wa_{name}"], d[f"wb_{name}"] = wx, wa, wb
    d["fr_1"], d["frt_1"] = _final_rhs(params, "l33")
    d["fr_2"], d["frt_2"] = _final_rhs(params, "l44")
    for k, (wk, bk) in [("1", ("l2W", "l2b")), ("2", ("l5W", "l5b"))]:
        w = np.asarray(params[wk], F32)  # [256, 256]
        b = np.asarray(params[bk], F32)  # [256]
        d[f"h1rhs_{k}"] = np.concatenate([w.T[0:128], w.T[128:256]], axis=1).reshape(
            128, 512
        ).copy()
        # chunk j of rhs = h1rhs[:, j*256:(j+1)*256]  (rows j*128..j*128+128 of w.T)
        d[f"h1rhs_{k}"] = np.concatenate(
            [w.T[0:128], w.T[128:256]], axis=1
        ).astype(F32)  # [128, 512]: [:, :256] = w.T rows 0:128
        d[f"h1b_{k}"] = b[None, :].astype(F32)  # [1, 256]
    d["l3rep_1"] = np.broadcast_to(
        np.asarray(params["l3W"], F32).reshape(1, 256), (128, 256)
    ).copy()
    d["l3rep_2"] = np.broadcast_to(
        np.asarray(params["l6W"], F32).reshape(1, 256), (128, 256)
    ).copy()
    d["ident"] = np.eye(128, dtype=F32)
    return d


def _prep_core(c, xa, h1_, c1_, h2_, c2_):
    """Per-core data tensors. xa: [B, T, 16] full concat(state, action)."""
    sl = slice(c * BL, (c + 1) * BL)
    xa_c = xa[sl]  # [128, 64, 16]
    d = {}
    xT = np.empty((KX, TS * BL), F32)
    # x_T[i, t*128 + b] = xa_c[b, t+1, i];  ones row at i=16
    xT[:IN] = xa_c[:, 1:, :].transpose(2, 1, 0).reshape(IN, TS * BL)
    xT[IN] = 1.0
    d["xT"] = xT
    s0 = xa_c[:, 1:, 0] / LONG  # [128, 63]
    s1 = xa_c[:, 1:, 1]
    d["s0s1"] = np.concatenate([s0, s1], axis=1).astype(F32)  # [128, 126]
    for name, h in [("hT1", h1_), ("hT2", h2_)]:
        hseg = h[sl]  # [128, 256]
        d[name] = np.concatenate(
            [hseg[:, 0:128].T, hseg[:, 128:256].T], axis=1
        ).astype(F32)  # [128, 256]
    d["c1b"] = c1_[sl].astype(F32)
    d["c2b"] = c2_[sl].astype(F32)
    S0sum = (xa_c[:, :, 0] / LONG).sum(axis=1)  # [128], over ALL 64 steps
    S1sum = xa_c[:, :, 1].sum(axis=1)
    d["extras"] = np.stack(
        [np.ones(BL, F32), S0sum.astype(F32), S1sum.astype(F32)]
    )  # [3, 128]
    return d


def _host_prep(state, action, h1, c1, h2, c2, params):
    state = np.asarray(state, F32)
    action = np.asarray(action, F32)
    st = state.reshape(B, T, SD)
    na = np.broadcast_to(action.reshape(B, 1, A), (B, T, A))
    xa = np.concatenate([st, na], axis=-1).astype(F32)  # [B, T, 16]
    shared = _prep_shared(params)
    h1_, c1_ = np.asarray(h1, F32)[0], np.asarray(c1, F32)[0]
    h2_, c2_ = np.asarray(h2, F32)[0], np.asarray(c2, F32)[0]
    cores = [_prep_core(c, xa, h1_, c1_, h2_, c2_) for c in range(NCORES)]
    l3b = (
        float(np.asarray(params["l3b"], F32).reshape(-1)[0]),
        float(np.asarray(params["l6b"], F32).reshape(-1)[0]),
    )
    return shared, cores, l3b


# ---------------------------------------------------------------------------
# numpy model of the EXACT device decomposition (for validating the algebra)
# ---------------------------------------------------------------------------


def _np_sig(x):
    return 1.0 / (1.0 + np.exp(-x))


def decomp_reference(state, action, h1, c1, h2, c2, params):
    shared, cores, l3b = _host_prep(state, action, h1, c1, h2, c2, params)
    q1o, q2o = [], []
    for c in range(NCORES):
        cd = cores[c]
        xT = cd["xT"].reshape(KX, TS, BL)
        s0 = cd["s0s1"][:, :TS]
        s1 = cd["s0s1"][:, TS:]

        def gates(hT, enc):
            # hT: [128part(h), 256]; returns list per t? helper for one t
            pass

        def step_gates(xcol, hT, enc):
            # xcol: [17, 128]; hT: [128, 256] (chunks side by side)
            lhs = np.concatenate(
                [xcol, hT[:, 0:128], hT[:, 128:256]], axis=0
            )  # [273, 128]
            rhs = np.concatenate(
                [shared[f"wx_{enc}"], shared[f"wa_{enc}"], shared[f"wb_{enc}"]],
                axis=0,
            )  # [273, 1024]
            return lhs.T @ rhs  # [128, 1024]

        def gate_apply(g, cprev):
            sig = _np_sig(g[:, 0:768])
            tg = np.tanh(g[:, 768:1024])
            cn = sig[:, 0:256] * tg + sig[:, 256:512] * cprev
            hn = sig[:, 512:768] * np.tanh(cn)
            return hn, cn

        # serial encoders
        sh_T = {}
        for enc, hkey, ckey in [("e1", "hT1", "c1b"), ("e2", "hT2", "c2b")]:
            hT = cd[hkey].copy()
            cb = cd[ckey].copy()
            for j in range(TS):
                t = TS - 1 - j
                g = step_gates(xT[:, t], hT, enc)
                hn, cb = gate_apply(g, cb)
                hT = np.concatenate([hn[:, 0:128].T, hn[:, 128:256].T], axis=1)
            sh_T[enc] = hT

        # parallel encoders: diag-weighted sums of h_t
        uv_T = {}
        for enc, hkey, ckey in [("e3", "hT1", "c1b"), ("e4", "hT2", "c2b")]:
            acc0 = np.zeros((BL, H), F32)
            acc1 = np.zeros((BL, H), F32)
            for t in range(TS):
                g = step_gates(xT[:, t], cd[hkey], enc)
                hn, _ = gate_apply(g, cd[ckey])
                acc0 += np.diag(s0[:, t]) @ hn
                acc1 += np.diag(s1[:, t]) @ hn
            uv_T[enc] = np.concatenate(
                [
                    acc0[:, 0:128].T,
                    acc0[:, 128:256].T,
                    acc1[:, 0:128].T,
                    acc1[:, 128:256].T,
                ],
                axis=1,
            )  # [128, 512]

        # final per critic
        for k, senc, penc, hkey, ckey in [
            ("1", "e1", "e3", "hT1", "c1b"),
            ("2", "e2", "e4", "hT2", "c2b"),
        ]:
            lhs = np.concatenate(
                [
                    sh_T[senc][:, 0:128],
                    sh_T[senc][:, 128:256],
                    uv_T[penc][:, 0:128],
                    uv_T[penc][:, 128:256],
                    uv_T[penc][:, 256:384],
                    uv_T[penc][:, 384:512],
                    cd[hkey][:, 0:128],
                    cd[hkey][:, 128:256],
                ],
                axis=1,
            )  # [128, 8*128]
            g = np.zeros((BL, G), F32)
            fr = shared[f"fr_{k}"]
            for ch in range(8):
                g += lhs[:, ch * 128 : (ch + 1) * 128].T @ fr[:, ch * G : (ch + 1) * G]
            g += cd["extras"].T @ shared[f"frt_{k}"]
            qh, _ = gate_apply(g, cd[ckey])
            # head
            h1rhs = shared[f"h1rhs_{k}"]
            qhT0 = qh[:, 0:128].T
            qhT1 = qh[:, 128:256].T
            hd = (
                qhT0.T @ h1rhs[:, 0:256]
                + qhT1.T @ h1rhs[:, 256:512]
                + np.ones((BL, 1)) @ shared[f"h1b_{k}"]
            )
            r1 = np.maximum(hd, 0.0)
            q = (r1 * shared[f"l3rep_{k}"]).sum(axis=1) + l3b[int(k) - 1]
            (q1o if k == "1" else q2o).append(q)
    q1 = np.concatenate(q1o).reshape(B, 1, 1)
    q2 = np.concatenate(q2o).reshape(B, 1, 1)
    return q1, q2


# ---------------------------------------------------------------------------
# Bass program
# ---------------------------------------------------------------------------

_SHARED_SPECS = [
    ("wx_e1", [KX, G]), ("wa_e1", [128, G]), ("wb_e1", [128, G]),
    ("wx_e2", [KX, G]), ("wa_e2", [128, G]), ("wb_e2", [128, G]),
    ("wx_e3", [KX, G]), ("wa_e3", [128, G]), ("wb_e3", [128, G]),
    ("wx_e4", [KX, G]), ("wa_e4", [128, G]), ("wb_e4", [128, G]),
    ("frt_1", [3, G]),
    ("frt_2", [3, G]),
    ("h1rhs_1", [128, 512]), ("h1b_1", [1, 256]),
    ("h1rhs_2", [128, 512]), ("h1b_2", [1, 256]),
    ("l3rep_1", [128, 256]), ("l3rep_2", [128, 256]),
    ("ident", [128, 128]),
]
_CORE_SPECS = [
    ("xT", [KX, TS * BL]),
    ("s0s1", [128, 2 * TS]),
    ("hT1", [128, 256]), ("hT2", [128, 256]),
    ("c1b", [128, 256]), ("c2b", [128, 256]),
    ("extras", [3, 128]),
]
# fr_1 / fr_2 [128, 8*G] stay as separate dram params, streamed at the end.

# Pack all small consts into 4 blobs keyed by partition count so the whole
# const load is 4 DMA instructions (per-instruction sync-wait limit).
_BLOB_PARTS = (128, KX, 3, 1)


def _blob_layout():
    offs = {}
    totals = {p: 0 for p in _BLOB_PARTS}
    for name, (p, cols) in _SHARED_SPECS + _CORE_SPECS:
        offs[name] = (p, totals[p], cols)
        totals[p] += cols
    return offs, totals


def _pack_blobs(tensors):
    offs, totals = _blob_layout()
    blobs = {p: np.zeros((p, totals[p]), F32) for p in _BLOB_PARTS if totals[p]}
    for name, arr in tensors.items():
        p, off, cols = offs[name]
        blobs[p][:, off : off + cols] = arr
    return {f"blob{p}": b for p, b in blobs.items()}


def build_bass(l3b):
    import concourse.bass as bass
    import concourse.tile as tile
    from concourse import mybir
    from contextlib import ExitStack

    f32 = mybir.dt.float32
    AF = mybir.ActivationFunctionType
    nc = bass.Bass()

    offs, totals = _blob_layout()
    dp = {}
    for p in _BLOB_PARTS:
        if totals[p]:
            dp[f"blob{p}"] = nc.declare_dram_parameter(
                f"blob{p}", [p, totals[p]], f32, isOutput=False
            )
    for k in ("1", "2"):
        dp[f"fr_{k}"] = nc.declare_dram_parameter(
            f"fr_{k}", [128, 8 * G], f32, isOutput=False
        )
    q_out = {
        "1": nc.declare_dram_parameter("q1", [128, 1], f32, isOutput=True),
        "2": nc.declare_dram_parameter("q2", [128, 1], f32, isOutput=True),
    }

    with ExitStack() as ctx:
        tc = ctx.enter_context(tile.TileContext(nc))
        const = ctx.enter_context(tc.tile_pool(name="const", bufs=1))
        frpool = ctx.enter_context(tc.tile_pool(name="frs", bufs=3))
        work = ctx.enter_context(tc.tile_pool(name="work", bufs=2))
        state_p = ctx.enter_context(tc.tile_pool(name="state", bufs=2))
        psum_g = ctx.enter_context(tc.tile_pool(name="psum_g", bufs=2, space="PSUM"))
        psum_s = ctx.enter_context(tc.tile_pool(name="psum_s", bufs=2, space="PSUM"))
        psum_a = ctx.enter_context(tc.tile_pool(name="psum_a", bufs=1, space="PSUM"))

        # ---- load constants into SBUF via 4 blob DMAs
        blob_tiles = {}
        for p in _BLOB_PARTS:
            if totals[p]:
                t = const.tile([p, totals[p]], f32, tag=f"blob{p}")
                nc.sync.dma_start(out=t[:], in_=dp[f"blob{p}"][:])
                blob_tiles[p] = t
        sb = {}
        for name, (p, off, cols) in offs.items():
            sb[name] = blob_tiles[p][:, off : off + cols]

        ident = sb["ident"]

        def gate_mms(g_ps, xcol, hT, enc):
            """6 matmuls accumulating gates [128, 1024] into g_ps.

            The K=17 x-chunk must not lead the accumulation group: small-K
            matmuls lower through the row-tiling LDWEIGHTS path, which has
            fewer HW sync-wait slots, so it must carry at most one wait.
            """
            wx, wa, wb = sb[f"wx_{enc}"], sb[f"wa_{enc}"], sb[f"wb_{enc}"]
            for half in range(2):
                o = g_ps[:, half * 512 : (half + 1) * 512]
                nsl = slice(half * 512, (half + 1) * 512)
                nc.tensor.matmul(o, hT[:, 0:128], wa[:, nsl], start=True, stop=False)
                nc.tensor.matmul(o, hT[:, 128:256], wb[:, nsl], start=False, stop=False)
                nc.tensor.matmul(o, xcol, wx[:, nsl], start=False, stop=True)

        def gate_apply(g_ps, c_prev, c_new, h_new):
            """sigmoid/tanh + cell update. c_new/h_new are SBUF tiles [128,256].

            Sigmoid via 0.5*tanh(x/2)+0.5: the ACT sigmoid spline's error
            budget (40 ULP) gets amplified ~1e4x through the 63-step
            recurrence; the tanh table is 4 ULP.
            """
            sgr = work.tile([128, 768], f32, tag="sgr")
            nc.scalar.activation(sgr[:], g_ps[:, 0:768], AF.Tanh, scale=0.5)
            sig = work.tile([128, 768], f32, tag="sig")
            nc.vector.tensor_scalar(
                sig[:], sgr[:], 0.5, 0.5,
                mybir.AluOpType.mult, mybir.AluOpType.add,
            )
            tg = work.tile([128, 256], f32, tag="tg")
            nc.scalar.activation(tg[:], g_ps[:, 768:1024], AF.Tanh)
            t1 = work.tile([128, 256], f32, tag="t1")
            nc.vector.tensor_mul(t1[:], sig[:, 0:256], tg[:])
            t2 = work.tile([128, 256], f32, tag="t2")
            nc.vector.tensor_mul(t2[:], sig[:, 256:512], c_prev[:])
            nc.vector.tensor_add(c_new[:], t1[:], t2[:])
            tcc = work.tile([128, 256], f32, tag="tcc")
            nc.scalar.activation(tcc[:], c_new[:], AF.Tanh)
            nc.vector.tensor_mul(h_new[:], sig[:, 512:768], tcc[:])

        def transpose256(dst_sbuf, src_sbuf, tag="tp"):
            """dst[128,256] = per-128-chunk transpose of src [128,256]."""
            tp = psum_s.tile([128, 256], f32, tag="tp_ps")
            nc.tensor.transpose(tp[:, 0:128], src_sbuf[:, 0:128], ident[:])
            nc.tensor.transpose(tp[:, 128:256], src_sbuf[:, 128:256], ident[:])
            nc.scalar.copy(dst_sbuf[:], tp[:])

        # ================= serial encoders (interleaved) =================
        ser = {}
        for enc, hkey, ckey in [("e1", "hT1", "c1b"), ("e2", "hT2", "c2b")]:
            ser[enc] = {"hT": sb[hkey], "c": sb[ckey]}
        for j in range(TS):
            t = TS - 1 - j
            for enc in ("e1", "e2"):
                st_ = ser[enc]
                g_ps = psum_g.tile([128, G], f32, tag="gates")
                xcol = sb["xT"][:, t * BL : (t + 1) * BL]
                gate_mms(g_ps, xcol, st_["hT"], enc)
                c_new = state_p.tile([128, 256], f32, tag=f"c_{enc}")
                h_new = work.tile([128, 256], f32, tag=f"h_{enc}")
                gate_apply(g_ps, st_["c"], c_new, h_new)
                hT_new = state_p.tile([128, 256], f32, tag=f"hT_{enc}")
                transpose256(hT_new, h_new)
                st_["hT"], st_["c"] = hT_new, c_new

        # ================= parallel encoders =================
        uvT = {}
        for enc, hkey, ckey in [("e3", "hT1", "c1b"), ("e4", "hT2", "c2b")]:
            acc = psum_a.tile([128, 512], f32, tag="acc")
            c0 = sb[ckey]
            hT0 = sb[hkey]
            for t in range(TS):
                g_ps = psum_g.tile([128, G], f32, tag="gates")
                xcol = sb["xT"][:, t * BL : (t + 1) * BL]
                gate_mms(g_ps, xcol, hT0, enc)
                c_new = work.tile([128, 256], f32, tag="cpar")
                h_new = work.tile([128, 256], f32, tag="hpar")
                gate_apply(g_ps, c0, c_new, h_new)
                dg0 = work.tile([128, 128], f32, tag="dg0")
                nc.vector.tensor_scalar_mul(dg0[:], ident[:], sb["s0s1"][:, t : t + 1])
                dg1 = work.tile([128, 128], f32, tag="dg1")
                nc.vector.tensor_scalar_mul(
                    dg1[:], ident[:], sb["s0s1"][:, TS + t : TS + t + 1]
                )
                nc.tensor.matmul(
                    acc[:, 0:256], dg0[:], h_new[:],
                    start=(t == 0), stop=(t == TS - 1), skip_group_check=True,
                )
                nc.tensor.matmul(
                    acc[:, 256:512], dg1[:], h_new[:],
                    start=(t == 0), stop=(t == TS - 1), skip_group_check=True,
                )
            accs = work.tile([128, 512], f32, tag="accs")
            nc.scalar.copy(accs[:], acc[:])
            uvT_e = const.tile([128, 512], f32, tag=f"uvT_{enc}")
            tp = psum_s.tile([128, 256], f32, tag="tp_ps")
            nc.tensor.transpose(tp[:, 0:128], accs[:, 0:128], ident[:])
            nc.tensor.transpose(tp[:, 128:256], accs[:, 128:256], ident[:])
            nc.scalar.copy(uvT_e[:, 0:256], tp[:])
            tp2 = psum_s.tile([128, 256], f32, tag="tp_ps")
            nc.tensor.transpose(tp2[:, 0:128], accs[:, 256:384], ident[:])
            nc.tensor.transpose(tp2[:, 128:256], accs[:, 384:512], ident[:])
            nc.scalar.copy(uvT_e[:, 256:512], tp2[:])
            uvT[enc] = uvT_e

        # ================= final LSTM + heads =================
        for k, senc, penc, hkey, ckey in [
            ("1", "e1", "e3", "hT1", "c1b"),
            ("2", "e2", "e4", "hT2", "c2b"),
        ]:
            g_ps = psum_g.tile([128, G], f32, tag="gates")
            sh_T = ser[senc]["hT"]
            uv = uvT[penc]
            chunks = [
                sh_T[:, 0:128], sh_T[:, 128:256],
                uv[:, 0:128], uv[:, 128:256], uv[:, 256:384], uv[:, 384:512],
                sb[hkey][:, 0:128], sb[hkey][:, 128:256],
            ]
            frt = sb[f"frt_{k}"]
            for ci, ch in enumerate(chunks):
                fr_c = frpool.tile([128, G], f32, tag="frch")
                nc.sync.dma_start(
                    out=fr_c[:], in_=dp[f"fr_{k}"][:, ci * G : (ci + 1) * G]
                )
                for half in range(2):
                    nc.tensor.matmul(
                        g_ps[:, half * 512 : (half + 1) * 512],
                        ch, fr_c[:, half * 512 : (half + 1) * 512],
                        start=(ci == 0), stop=False,
                    )
            for half in range(2):
                nc.tensor.matmul(
                    g_ps[:, half * 512 : (half + 1) * 512],
                    sb["extras"][:], frt[:, half * 512 : (half + 1) * 512],
                    start=False, stop=True,
                )
            c_new = work.tile([128, 256], f32, tag="cfin")
            qh = work.tile([128, 256], f32, tag="qh")
            gate_apply(g_ps, sb[ckey], c_new, qh)
            qhT = work.tile([128, 256], f32, tag="qhT")
            transpose256(qhT, qh)
            hd_ps = psum_s.tile([128, 256], f32, tag="tp_ps")
            nc.tensor.matmul(
                hd_ps[:], qhT[:, 0:128], sb[f"h1rhs_{k}"][:, 0:256],
                start=True, stop=False,
            )
            nc.tensor.matmul(
                hd_ps[:], qhT[:, 128:256], sb[f"h1rhs_{k}"][:, 256:512],
                start=False, stop=False,
            )
            nc.tensor.matmul(
                hd_ps[:], sb["extras"][0:1, :], sb[f"h1b_{k}"][:],
                start=False, stop=True,
            )
            r1 = work.tile([128, 256], f32, tag="r1")
            nc.scalar.activation(r1[:], hd_ps[:], AF.Relu)
            m = work.tile([128, 256], f32, tag="m")
            nc.vector.tensor_mul(m[:], r1[:], sb[f"l3rep_{k}"][:])
            qv = work.tile([128, 1], f32, tag="qv")
            nc.vector.tensor_reduce(
                qv[:], m[:], mybir.AxisListType.X, mybir.AluOpType.add
            )
            qf = work.tile([128, 1], f32, tag="qf")
            nc.vector.tensor_scalar_add(qf[:], qv[:], float(l3b[int(k) - 1]))
            nc.sync.dma_start(out=q_out[k][:], in_=qf[:])

    _split_multi_waits(nc)
    return nc


def _split_multi_waits(nc):
    """This toolchain's walrus accepts at most ONE semaphore wait per TPB
    instruction ("Too many sync wait commands"). Tile emits up to 3. Split:
    hoist extra waits onto same-engine NoOps placed just before the
    instruction — NX dispatch is in-order per engine, so a NoOp's wait
    gates everything behind it on that engine.
    """
    from concourse import mybir

    n_split = 0
    for fn in nc.m.functions:
        for blk in fn.blocks:
            insts = list(blk.instructions)
            new = []
            for ins in insts:
                si = getattr(ins, "sync_info", None)
                waits = list(si.on_wait) if si is not None and si.on_wait else []
                if len(waits) > 1:
                    for j, w in enumerate(waits[:-1]):
                        new.append(
                            mybir.InstNoOp(
                                name=f"{ins.name}-w{j}",
                                ins=[],
                                outs=[],
                                engine=ins.engine,
                                sync_info=mybir.SyncInfo(on_wait=[w], on_update=[]),
                            )
                        )
                        n_split += 1
                    ins.sync_info = mybir.SyncInfo(
                        on_wait=[waits[-1]], on_update=list(si.on_update or [])
                    )
                new.append(ins)
            if n_split:
                try:
                    blk.instructions = new
                except Exception:
                    blk.instructions.clear()
                    blk.instructions.extend(new)
    return n_split


_BASS_CACHE = {}


def kernel(state, action, h1, c1, h2, c2, params):
    from concourse.bass_utils import run_bass_kernel_spmd

    shared, cores, l3b = _host_prep(state, action, h1, c1, h2, c2, params)

    key = "k"
    if key not in _BASS_CACHE:
        _BASS_CACHE[key] = build_bass(l3b)
    nc = _BASS_CACHE[key]

    fr = {k: shared.pop(k) for k in ("fr_1", "fr_2")}
    in_maps = []
    for c in range(NCORES):
        m = _pack_blobs({**shared, **cores[c]})
        m.update(fr)
        in_maps.append(m)

    trace = bool(int(os.environ.get("KERNEL_TRACE", "0")))
    res = run_bass_kernel_spmd(nc, in_maps, list(range(NCORES)), trace=trace)
    global last_exec_time_ns
    last_exec_time_ns = res.exec_time_ns

    q1 = np.concatenate([res.results[c]["q1"].reshape(BL) for c in range(NCORES)])
    q2 = np.concatenate([res.results[c]["q2"].reshape(BL) for c in range(NCORES)])
    return q1.reshape(B, 1, 1).astype(F32), q2.reshape(B, 1, 1).astype(F32)


last_exec_time_ns = None


# revision 20
# speedup vs baseline: 1.1594x; 1.1594x over previous
"""Trainium2 Bass kernel for nn_Critic (twin-critic LSTM network).

Strategy: pure data parallelism — batch B=1024 split as 128 per core across
8 NeuronCores (128 = SBUF partition count, so batch maps to partitions).

Decomposition (validated in numpy by `decomp_reference`):
  * gate rows reordered (i,f,g,o) -> (i,f,o,g) so one sigmoid covers [0:768]
  * serial LSTMs: per-step gates = lhsT.T @ rhs with
      lhsT = [x_aug_t (17 = 7 state + 9 action + ones) ; h_T (256)]   (K chunks 17/128/128)
      rhs  = [Wih_aug^T ; Whh^T]  (bias folded into the ones row)
    h kept transposed on-chip via PE transpose each step.
  * parallel encoders: same gate matmul with h_T fixed at h0_T; only
    fuse(h_all) is used by the reference, and
      sum_t (z @ f1W.T + f1b) * s0  ==  (sum_t s0_t z_t) @ f1W.T + f1b * S0sum
    The weighted time-sum is computed on PE as diag(s_t) matmuls
    accumulating in PSUM.
  * fuse linears + concat + final LSTM input proj fold into ONE matmul with
    host-precomputed combined weights (1027 contraction rows incl. rank-1
    bias terms via S0sum/S1sum rows).
"""

import os
import sys

import numpy as np

sys.path.insert(0, "/opt/trn_rl_repo")

B, T, A, H, SD = 1024, 64, 9, 256, 7
IN = SD + A  # 16
LONG = 10.0
NCORES = 8
BL = B // NCORES  # 128
TS = T - 1  # 63
G = 4 * H  # 1024
KX = IN + 1  # 17: x features + ones row

F32 = np.float32


def _perm():
    # (i,f,g,o) -> (i,f,o,g)
    return np.r_[0:512, 768:1024, 512:768]


def _enc_weights(p):
    """rhs tensors for one encoder: wx_aug [17,1024], wa [128,1024], wb [128,1024]."""
    perm = _perm()
    wih = np.asarray(p["Wih"], F32)[perm]  # [1024, 16]
    whh = np.asarray(p["Whh"], F32)[perm]  # [1024, 256]
    bias = (np.asarray(p["bih"], F32) + np.asarray(p["bhh"], F32))[perm]  # [1024]
    wx_aug = np.concatenate([wih.T, bias[None, :]], axis=0).astype(F32)  # [17, 1024]
    wa = whh.T[0:128].copy()
    wb = whh.T[128:256].copy()
    return wx_aug, wa, wb


def _final_rhs(params, enc_key):
    """Final-LSTM folded rhs for one critic: fr [128, 8*1024], frt [3, 1024]."""
    perm = _perm()
    L = params["L"]
    wih_L = np.asarray(L["Wih"], F32)[perm]  # [1024, 512]
    whh_L = np.asarray(L["Whh"], F32)[perm]  # [1024, 256]
    bias_L = (np.asarray(L["bih"], F32) + np.asarray(L["bhh"], F32))[perm]  # [1024]
    P1 = wih_L[:, 256:384]  # [1024, 128]
    P2 = wih_L[:, 384:512]
    enc = params[enc_key]
    f1W = np.asarray(enc["f1W"], F32)  # [128, 256]
    f2W = np.asarray(enc["f2W"], F32)
    f1b = np.asarray(enc["f1b"], F32)  # [128]
    f2b = np.asarray(enc["f2b"], F32)
    M1 = P1 @ f1W  # [1024, 256]
    M2 = P2 @ f2W
    rows = np.concatenate(
        [wih_L[:, 0:256].T, M1.T, M2.T, whh_L.T], axis=0
    ).astype(F32)  # [1024, 1024]
    fr = rows.reshape(8, 128, G).transpose(1, 0, 2).reshape(128, 8 * G).copy()
    frt = np.stack([bias_L, P1 @ f1b, P2 @ f2b]).astype(F32)  # [3, 1024]
    return fr, frt


def _prep_shared(params):
    d = {}
    for name, key in [("e1", "l11"), ("e2", "l22"), ("e3", "l33"), ("e4", "l44")]:
        wx, wa, wb = _enc_weights(params[key])
        d[f"wx_{name}"], d[f"wa_{name}"], d[f"wb_{name}"] = wx, wa, wb
    d["fr_1"], d["frt_1"] = _final_rhs(params, "l33")
    d["fr_2"], d["frt_2"] = _final_rhs(params, "l44")
    for k, (wk, bk) in [("1", ("l2W", "l2b")), ("2", ("l5W", "l5b"))]:
        w = np.asarray(params[wk], F32)  # [256, 256]
        b = np.asarray(params[bk], F32)  # [256]
        d[f"h1rhs_{k}"] = np.concatenate([w.T[0:128], w.T[128:256]], axis=1).reshape(
            128, 512
        ).copy()
        # chunk j of rhs = h1rhs[:, j*256:(j+1)*256]  (rows j*128..j*128+128 of w.T)
        d[f"h1rhs_{k}"] = np.concatenate(
            [w.T[0:128], w.T[128:256]], axis=1
        ).astype(F32)  # [128, 512]: [:, :256] = w.T rows 0:128
        d[f"h1b_{k}"] = b[None, :].astype(F32)  # [1, 256]
    d["l3rep_1"] = np.broadcast_to(
        np.asarray(params["l3W"], F32).reshape(1, 256), (128, 256)
    ).copy()
    d["l3rep_2"] = np.broadcast_to(
        np.asarray(params["l6W"], F32).reshape(1, 256), (128, 256)
    ).copy()
    d["ident"] = np.eye(128, dtype=F32)
    return d


def _prep_core(c, xa, h1_, c1_, h2_, c2_):
    """Per-core data tensors. xa: [B, T, 16] full concat(state, action)."""
    sl = slice(c * BL, (c + 1) * BL)
    xa_c = xa[sl]  # [128, 64, 16]
    d = {}
    xT = np.empty((KX, TS * BL), F32)
    # x_T[i, t*128 + b] = xa_c[b, t+1, i];  ones row at i=16
    xT[:IN] = xa_c[:, 1:, :].transpose(2, 1, 0).reshape(IN, TS * BL)
    xT[IN] = 1.0
    d["xT"] = xT
    s0 = xa_c[:, 1:, 0] / LONG  # [128, 63]
    s1 = xa_c[:, 1:, 1]
    d["s0s1"] = np.concatenate([s0, s1], axis=1).astype(F32)  # [128, 126]
    for name, h in [("hT1", h1_), ("hT2", h2_)]:
        hseg = h[sl]  # [128, 256]
        d[name] = np.concatenate(
            [hseg[:, 0:128].T, hseg[:, 128:256].T], axis=1
        ).astype(F32)  # [128, 256]
    d["c1b"] = c1_[sl].astype(F32)
    d["c2b"] = c2_[sl].astype(F32)
    S0sum = (xa_c[:, :, 0] / LONG).sum(axis=1)  # [128], over ALL 64 steps
    S1sum = xa_c[:, :, 1].sum(axis=1)
    d["extras"] = np.stack(
        [np.ones(BL, F32), S0sum.astype(F32), S1sum.astype(F32)]
    )  # [3, 128]
    return d


def _host_prep(state, action, h1, c1, h2, c2, params):
    state = np.asarray(state, F32)
    action = np.asarray(action, F32)
    st = state.reshape(B, T, SD)
    na = np.broadcast_to(action.reshape(B, 1, A), (B, T, A))
    xa = np.concatenate([st, na], axis=-1).astype(F32)  # [B, T, 16]
    shared = _prep_shared(params)
    h1_, c1_ = np.asarray(h1, F32)[0], np.asarray(c1, F32)[0]
    h2_, c2_ = np.asarray(h2, F32)[0], np.asarray(c2, F32)[0]
    cores = [_prep_core(c, xa, h1_, c1_, h2_, c2_) for c in range(NCORES)]
    l3b = (
        float(np.asarray(params["l3b"], F32).reshape(-1)[0]),
        float(np.asarray(params["l6b"], F32).reshape(-1)[0]),
    )
    return shared, cores, l3b


# ---------------------------------------------------------------------------
# numpy model of the EXACT device decomposition (for validating the algebra)
# ---------------------------------------------------------------------------


def _np_sig(x):
    return 1.0 / (1.0 + np.exp(-x))


def decomp_reference(state, action, h1, c1, h2, c2, params):
    shared, cores, l3b = _host_prep(state, action, h1, c1, h2, c2, params)
    q1o, q2o = [], []
    for c in range(NCORES):
        cd = cores[c]
        xT = cd["xT"].reshape(KX, TS, BL)
        s0 = cd["s0s1"][:, :TS]
        s1 = cd["s0s1"][:, TS:]

        def gates(hT, enc):
            # hT: [128part(h), 256]; returns list per t? helper for one t
            pass

        def step_gates(xcol, hT, enc):
            # xcol: [17, 128]; hT: [128, 256] (chunks side by side)
            lhs = np.concatenate(
                [xcol, hT[:, 0:128], hT[:, 128:256]], axis=0
            )  # [273, 128]
            rhs = np.concatenate(
                [shared[f"wx_{enc}"], shared[f"wa_{enc}"], shared[f"wb_{enc}"]],
                axis=0,
            )  # [273, 1024]
            return lhs.T @ rhs  # [128, 1024]

        def gate_apply(g, cprev):
            sig = _np_sig(g[:, 0:768])
            tg = np.tanh(g[:, 768:1024])
            cn = sig[:, 0:256] * tg + sig[:, 256:512] * cprev
            hn = sig[:, 512:768] * np.tanh(cn)
            return hn, cn

        # serial encoders
        sh_T = {}
        for enc, hkey, ckey in [("e1", "hT1", "c1b"), ("e2", "hT2", "c2b")]:
            hT = cd[hkey].copy()
            cb = cd[ckey].copy()
            for j in range(TS):
                t = TS - 1 - j
                g = step_gates(xT[:, t], hT, enc)
                hn, cb = gate_apply(g, cb)
                hT = np.concatenate([hn[:, 0:128].T, hn[:, 128:256].T], axis=1)
            sh_T[enc] = hT

        # parallel encoders: diag-weighted sums of h_t
        uv_T = {}
        for enc, hkey, ckey in [("e3", "hT1", "c1b"), ("e4", "hT2", "c2b")]:
            acc0 = np.zeros((BL, H), F32)
            acc1 = np.zeros((BL, H), F32)
            for t in range(TS):
                g = step_gates(xT[:, t], cd[hkey], enc)
                hn, _ = gate_apply(g, cd[ckey])
                acc0 += np.diag(s0[:, t]) @ hn
                acc1 += np.diag(s1[:, t]) @ hn
            uv_T[enc] = np.concatenate(
                [
                    acc0[:, 0:128].T,
                    acc0[:, 128:256].T,
                    acc1[:, 0:128].T,
                    acc1[:, 128:256].T,
                ],
                axis=1,
            )  # [128, 512]

        # final per critic
        for k, senc, penc, hkey, ckey in [
            ("1", "e1", "e3", "hT1", "c1b"),
            ("2", "e2", "e4", "hT2", "c2b"),
        ]:
            lhs = np.concatenate(
                [
                    sh_T[senc][:, 0:128],
                    sh_T[senc][:, 128:256],
                    uv_T[penc][:, 0:128],
                    uv_T[penc][:, 128:256],
                    uv_T[penc][:, 256:384],
                    uv_T[penc][:, 384:512],
                    cd[hkey][:, 0:128],
                    cd[hkey][:, 128:256],
                ],
                axis=1,
            )  # [128, 8*128]
            g = np.zeros((BL, G), F32)
            fr = shared[f"fr_{k}"]
            for ch in range(8):
                g += lhs[:, ch * 128 : (ch + 1) * 128].T @ fr[:, ch * G : (ch + 1) * G]
            g += cd["extras"].T @ shared[f"frt_{k}"]
            qh, _ = gate_apply(g, cd[ckey])
            # head
            h1rhs = shared[f"h1rhs_{k}"]
            qhT0 = qh[:, 0:128].T
            qhT1 = qh[:, 128:256].T
            hd = (
                qhT0.T @ h1rhs[:, 0:256]
                + qhT1.T @ h1rhs[:, 256:512]
                + np.ones((BL, 1)) @ shared[f"h1b_{k}"]
            )
            r1 = np.maximum(hd, 0.0)
            q = (r1 * shared[f"l3rep_{k}"]).sum(axis=1) + l3b[int(k) - 1]
            (q1o if k == "1" else q2o).append(q)
    q1 = np.concatenate(q1o).reshape(B, 1, 1)
    q2 = np.concatenate(q2o).reshape(B, 1, 1)
    return q1, q2


# ---------------------------------------------------------------------------
# Bass program
# ---------------------------------------------------------------------------

_SHARED_SPECS = [
    ("wx_e1", [KX, G]), ("wa_e1", [128, G]), ("wb_e1", [128, G]),
    ("wx_e2", [KX, G]), ("wa_e2", [128, G]), ("wb_e2", [128, G]),
    ("wx_e3", [KX, G]), ("wa_e3", [128, G]), ("wb_e3", [128, G]),
    ("wx_e4", [KX, G]), ("wa_e4", [128, G]), ("wb_e4", [128, G]),
    ("frt_1", [3, G]),
    ("frt_2", [3, G]),
    ("h1rhs_1", [128, 512]), ("h1b_1", [1, 256]),
    ("h1rhs_2", [128, 512]), ("h1b_2", [1, 256]),
    ("l3rep_1", [128, 256]), ("l3rep_2", [128, 256]),
    ("ident", [128, 128]),
]
_CORE_SPECS = [
    ("xT", [KX, TS * BL]),
    ("s0s1", [128, 2 * TS]),
    ("hT1", [128, 256]), ("hT2", [128, 256]),
    ("c1b", [128, 256]), ("c2b", [128, 256]),
    ("extras", [3, 128]),
]
# fr_1 / fr_2 [128, 8*G] stay as separate dram params, streamed at the end.

# Pack all small consts into 4 blobs keyed by partition count so the whole
# const load is 4 DMA instructions (per-instruction sync-wait limit).
_BLOB_PARTS = (128, KX, 3, 1)


def _blob_layout():
    offs = {}
    totals = {p: 0 for p in _BLOB_PARTS}
    for name, (p, cols) in _SHARED_SPECS + _CORE_SPECS:
        offs[name] = (p, totals[p], cols)
        totals[p] += cols
    return offs, totals


def _pack_blobs(tensors):
    offs, totals = _blob_layout()
    blobs = {p: np.zeros((p, totals[p]), F32) for p in _BLOB_PARTS if totals[p]}
    for name, arr in tensors.items():
        p, off, cols = offs[name]
        blobs[p][:, off : off + cols] = arr
    return {f"blob{p}": b for p, b in blobs.items()}


def build_bass(l3b):
    import concourse.bass as bass
    import concourse.tile as tile
    from concourse import mybir
    from contextlib import ExitStack

    f32 = mybir.dt.float32
    AF = mybir.ActivationFunctionType
    nc = bass.Bass()

    offs, totals = _blob_layout()
    dp = {}
    for p in _BLOB_PARTS:
        if totals[p]:
            dp[f"blob{p}"] = nc.declare_dram_parameter(
                f"blob{p}", [p, totals[p]], f32, isOutput=False
            )
    for k in ("1", "2"):
        dp[f"fr_{k}"] = nc.declare_dram_parameter(
            f"fr_{k}", [128, 8 * G], f32, isOutput=False
        )
    q_out = {
        "1": nc.declare_dram_parameter("q1", [128, 1], f32, isOutput=True),
        "2": nc.declare_dram_parameter("q2", [128, 1], f32, isOutput=True),
    }

    with ExitStack() as ctx:
        tc = ctx.enter_context(tile.TileContext(nc))
        const = ctx.enter_context(tc.tile_pool(name="const", bufs=1))
        frpool = ctx.enter_context(tc.tile_pool(name="frs", bufs=3))
        work = ctx.enter_context(tc.tile_pool(name="work", bufs=3))
        state_p = ctx.enter_context(tc.tile_pool(name="state", bufs=2))
        psum_g = ctx.enter_context(tc.tile_pool(name="psum_g", bufs=3, space="PSUM"))
        psum_s = ctx.enter_context(tc.tile_pool(name="psum_s", bufs=1, space="PSUM"))
        psum_a = ctx.enter_context(tc.tile_pool(name="psum_a", bufs=1, space="PSUM"))

        # ---- load constants into SBUF via 4 blob DMAs
        blob_tiles = {}
        for p in _BLOB_PARTS:
            if totals[p]:
                t = const.tile([p, totals[p]], f32, tag=f"blob{p}")
                nc.sync.dma_start(out=t[:], in_=dp[f"blob{p}"][:])
                blob_tiles[p] = t
        sb = {}
        for name, (p, off, cols) in offs.items():
            sb[name] = blob_tiles[p][:, off : off + cols]

        ident = sb["ident"]

        def gate_mms(g_ps, xcol, hT, enc):
            """6 matmuls accumulating gates [128, 1024] into g_ps.

            The K=17 x-chunk must not lead the accumulation group: small-K
            matmuls lower through the row-tiling LDWEIGHTS path, which has
            fewer HW sync-wait slots, so it must carry at most one wait.
            """
            wx, wa, wb = sb[f"wx_{enc}"], sb[f"wa_{enc}"], sb[f"wb_{enc}"]
            for half in range(2):
                o = g_ps[:, half * 512 : (half + 1) * 512]
                nsl = slice(half * 512, (half + 1) * 512)
                nc.tensor.matmul(o, hT[:, 0:128], wa[:, nsl], start=True, stop=False)
                nc.tensor.matmul(o, hT[:, 128:256], wb[:, nsl], start=False, stop=False)
                nc.tensor.matmul(o, xcol, wx[:, nsl], start=False, stop=True)

        def gate_apply(g_ps, c_prev, c_new, h_new):
            """sigmoid/tanh + cell update. c_new/h_new are SBUF tiles [128,256]."""
            sig = work.tile([128, 768], f32, tag="sig")
            nc.scalar.activation(sig[:], g_ps[:, 0:768], AF.Sigmoid)
            tg = work.tile([128, 256], f32, tag="tg")
            nc.scalar.activation(tg[:], g_ps[:, 768:1024], AF.Tanh)
            t1 = work.tile([128, 256], f32, tag="t1")
            nc.vector.tensor_mul(t1[:], sig[:, 0:256], tg[:])
            t2 = work.tile([128, 256], f32, tag="t2")
            nc.vector.tensor_mul(t2[:], sig[:, 256:512], c_prev[:])
            nc.vector.tensor_add(c_new[:], t1[:], t2[:])
            tcc = work.tile([128, 256], f32, tag="tcc")
            nc.scalar.activation(tcc[:], c_new[:], AF.Tanh)
            nc.vector.tensor_mul(h_new[:], sig[:, 512:768], tcc[:])

        def transpose256(dst_sbuf, src_sbuf, tag="tp"):
            """dst[128,256] = per-128-chunk transpose of src [128,256]."""
            tp = psum_s.tile([128, 256], f32, tag="tp_ps")
            nc.tensor.transpose(tp[:, 0:128], src_sbuf[:, 0:128], ident[:])
            nc.tensor.transpose(tp[:, 128:256], src_sbuf[:, 128:256], ident[:])
            nc.scalar.copy(dst_sbuf[:], tp[:])

        # ================= serial encoders (interleaved) =================
        ser = {}
        for enc, hkey, ckey in [("e1", "hT1", "c1b"), ("e2", "hT2", "c2b")]:
            ser[enc] = {"hT": sb[hkey], "c": sb[ckey]}
        for j in range(TS):
            t = TS - 1 - j
            for enc in ("e1", "e2"):
                st_ = ser[enc]
                g_ps = psum_g.tile([128, G], f32, tag="gates")
                xcol = sb["xT"][:, t * BL : (t + 1) * BL]
                gate_mms(g_ps, xcol, st_["hT"], enc)
                c_new = state_p.tile([128, 256], f32, tag=f"c_{enc}")
                h_new = work.tile([128, 256], f32, tag=f"h_{enc}")
                gate_apply(g_ps, st_["c"], c_new, h_new)
                hT_new = state_p.tile([128, 256], f32, tag=f"hT_{enc}")
                transpose256(hT_new, h_new)
                st_["hT"], st_["c"] = hT_new, c_new

        # ================= parallel encoders =================
        uvT = {}
        for enc, hkey, ckey in [("e3", "hT1", "c1b"), ("e4", "hT2", "c2b")]:
            acc = psum_a.tile([128, 512], f32, tag="acc")
            c0 = sb[ckey]
            hT0 = sb[hkey]
            for t in range(TS):
                g_ps = psum_g.tile([128, G], f32, tag="gates")
                xcol = sb["xT"][:, t * BL : (t + 1) * BL]
                gate_mms(g_ps, xcol, hT0, enc)
                c_new = work.tile([128, 256], f32, tag="cpar")
                h_new = work.tile([128, 256], f32, tag="hpar")
                gate_apply(g_ps, c0, c_new, h_new)
                dg0 = work.tile([128, 128], f32, tag="dg0")
                nc.vector.tensor_scalar_mul(dg0[:], ident[:], sb["s0s1"][:, t : t + 1])
                dg1 = work.tile([128, 128], f32, tag="dg1")
                nc.vector.tensor_scalar_mul(
                    dg1[:], ident[:], sb["s0s1"][:, TS + t : TS + t + 1]
                )
                nc.tensor.matmul(
                    acc[:, 0:256], dg0[:], h_new[:],
                    start=(t == 0), stop=(t == TS - 1), skip_group_check=True,
                )
                nc.tensor.matmul(
                    acc[:, 256:512], dg1[:], h_new[:],
                    start=(t == 0), stop=(t == TS - 1), skip_group_check=True,
                )
            accs = work.tile([128, 512], f32, tag="accs")
            nc.scalar.copy(accs[:], acc[:])
            uvT_e = const.tile([128, 512], f32, tag=f"uvT_{enc}")
            tp = psum_s.tile([128, 256], f32, tag="tp_ps")
            nc.tensor.transpose(tp[:, 0:128], accs[:, 0:128], ident[:])
            nc.tensor.transpose(tp[:, 128:256], accs[:, 128:256], ident[:])
            nc.scalar.copy(uvT_e[:, 0:256], tp[:])
            tp2 = psum_s.tile([128, 256], f32, tag="tp_ps")
            nc.tensor.transpose(tp2[:, 0:128], accs[:, 256:384], ident[:])
            nc.tensor.transpose(tp2[:, 128:256], accs[:, 384:512], ident[:])
            nc.scalar.copy(uvT_e[:, 256:512], tp2[:])
            uvT[enc] = uvT_e

        # ================= final LSTM + heads =================
        for k, senc, penc, hkey, ckey in [
            ("1", "e1", "e3", "hT1", "c1b"),
            ("2", "e2", "e4", "hT2", "c2b"),
        ]:
            g_ps = psum_g.tile([128, G], f32, tag="gates")
            sh_T = ser[senc]["hT"]
            uv = uvT[penc]
            chunks = [
                sh_T[:, 0:128], sh_T[:, 128:256],
                uv[:, 0:128], uv[:, 128:256], uv[:, 256:384], uv[:, 384:512],
                sb[hkey][:, 0:128], sb[hkey][:, 128:256],
            ]
            frt = sb[f"frt_{k}"]
            for ci, ch in enumerate(chunks):
                fr_c = frpool.tile([128, G], f32, tag="frch")
                nc.sync.dma_start(
                    out=fr_c[:], in_=dp[f"fr_{k}"][:, ci * G : (ci + 1) * G]
                )
                for half in range(2):
                    nc.tensor.matmul(
                        g_ps[:, half * 512 : (half + 1) * 512],
                        ch, fr_c[:, half * 512 : (half + 1) * 512],
                        start=(ci == 0), stop=False,
                    )
            for half in range(2):
                nc.tensor.matmul(
                    g_ps[:, half * 512 : (half + 1) * 512],
                    sb["extras"][:], frt[:, half * 512 : (half + 1) * 512],
                    start=False, stop=True,
                )
            c_new = work.tile([128, 256], f32, tag="cfin")
            qh = work.tile([128, 256], f32, tag="qh")
            gate_apply(g_ps, sb[ckey], c_new, qh)
            qhT = work.tile([128, 256], f32, tag="qhT")
            transpose256(qhT, qh)
            hd_ps = psum_s.tile([128, 256], f32, tag="tp_ps")
            nc.tensor.matmul(
                hd_ps[:], qhT[:, 0:128], sb[f"h1rhs_{k}"][:, 0:256],
                start=True, stop=False,
            )
            nc.tensor.matmul(
                hd_ps[:], qhT[:, 128:256], sb[f"h1rhs_{k}"][:, 256:512],
                start=False, stop=False,
            )
            nc.tensor.matmul(
                hd_ps[:], sb["extras"][0:1, :], sb[f"h1b_{k}"][:],
                start=False, stop=True,
            )
            r1 = work.tile([128, 256], f32, tag="r1")
            nc.scalar.activation(r1[:], hd_ps[:], AF.Relu)
            m = work.tile([128, 256], f32, tag="m")
            nc.vector.tensor_mul(m[:], r1[:], sb[f"l3rep_{k}"][:])
            qv = work.tile([128, 1], f32, tag="qv")
            nc.vector.tensor_reduce(
                qv[:], m[:], mybir.AxisListType.X, mybir.AluOpType.add
            )
            qf = work.tile([128, 1], f32, tag="qf")
            nc.vector.tensor_scalar_add(qf[:], qv[:], float(l3b[int(k) - 1]))
            nc.sync.dma_start(out=q_out[k][:], in_=qf[:])

    _split_multi_waits(nc)
    return nc


def _split_multi_waits(nc):
    """This toolchain's walrus accepts at most ONE semaphore wait per TPB
    instruction ("Too many sync wait commands"). Tile emits up to 3. Split:
    hoist extra waits onto same-engine NoOps placed just before the
    instruction — NX dispatch is in-order per engine, so a NoOp's wait
    gates everything behind it on that engine.
    """
    from concourse import mybir

    n_split = 0
    for fn in nc.m.functions:
        for blk in fn.blocks:
            insts = list(blk.instructions)
            new = []
            for ins in insts:
                si = getattr(ins, "sync_info", None)
                waits = list(si.on_wait) if si is not None and si.on_wait else []
                if len(waits) > 1:
                    for j, w in enumerate(waits[:-1]):
                        new.append(
                            mybir.InstNoOp(
                                name=f"{ins.name}-w{j}",
                                ins=[],
                                outs=[],
                                engine=ins.engine,
                                sync_info=mybir.SyncInfo(on_wait=[w], on_update=[]),
                            )
                        )
                        n_split += 1
                    ins.sync_info = mybir.SyncInfo(
                        on_wait=[waits[-1]], on_update=list(si.on_update or [])
                    )
                new.append(ins)
            if n_split:
                try:
                    blk.instructions = new
                except Exception:
                    blk.instructions.clear()
                    blk.instructions.extend(new)
    return n_split


_BASS_CACHE = {}


def kernel(state, action, h1, c1, h2, c2, params):
    from concourse.bass_utils import run_bass_kernel_spmd

    shared, cores, l3b = _host_prep(state, action, h1, c1, h2, c2, params)

    key = "k"
    if key not in _BASS_CACHE:
        _BASS_CACHE[key] = build_bass(l3b)
    nc = _BASS_CACHE[key]

    fr = {k: shared.pop(k) for k in ("fr_1", "fr_2")}
    in_maps = []
    for c in range(NCORES):
        m = _pack_blobs({**shared, **cores[c]})
        m.update(fr)
        in_maps.append(m)

    trace = bool(int(os.environ.get("KERNEL_TRACE", "0")))
    try:
        res = run_bass_kernel_spmd(nc, in_maps, list(range(NCORES)), trace=trace)
    except ModuleNotFoundError:
        # NTFF profile hook unavailable in this container - run untraced
        res = run_bass_kernel_spmd(nc, in_maps, list(range(NCORES)), trace=False)
    global last_exec_time_ns
    last_exec_time_ns = res.exec_time_ns

    q1 = np.concatenate([res.results[c]["q1"].reshape(BL) for c in range(NCORES)])
    q2 = np.concatenate([res.results[c]["q2"].reshape(BL) for c in range(NCORES)])
    return q1.reshape(B, 1, 1).astype(F32), q2.reshape(B, 1, 1).astype(F32)


last_exec_time_ns = None
